# revision 20
# baseline (speedup 1.0000x reference)
"""Trainium2 Bass kernel for BiDAF-style bidirectional attention.

Reference computation (per batch element n; M=1 folded away):
    s[i,j]  = h[i].w_h + u[j].w_u + (h[i]*u[j]).w_hu + b      [JX, JQ]
    a_u     = softmax_j(s);     u_a[i] = sum_j a_u[i,j] u[j]   (c2q)
    a_h     = softmax_i(max_j s);  h_a = sum_i a_h[i] h[i]     (q2c)
    out     = concat(h, u_a, h*u_a, h*h_a)                     [JX, 4D]

Sharding: data-parallel over batch N=8, one NeuronCore per batch element.
alpha_b drops out (both softmaxes are shift-invariant), accepted but unused.

Per-core I/O is 10.25 MiB (h 2 + u 0.25 + out 8), i.e. ~30 us at the 358 GB/s
per-core HBM roofline, so the schedule is built around keeping the store
stream continuous from ~4.5 us on:
  - 4 blocks of 2 i-tiles stream through scores->exp->c2q as h loads arrive
    (the c2q softmax over j is local to an i-row; only o4 = h*h_a needs the
    global max/softmax over all JX).
  - DMA count is minimized (each dma_start costs ~0.7 us of sequencer issue
    time): h as 4 x 512 KiB loads + u (sync), alpha_w broadcasts (gpsimd),
    o1 = h passthrough as 2 x 1 MiB bulk stores (gpsimd, SWDGE), per-block
    [o2|o3] slabs as 4 x 1 MiB stores and o4 as 2 x 1 MiB stores (sync).
  - f32 tiles are bitcast to f32r at matmul use sites (no cast copies).
  - scores are computed TRANSPOSED per block: sT[j,i] = sum_d uwT[d,j]hT[d,i]
    over 4 d-chunks, + h.w_h via a K=1 matmul (ones_col x hwh_row), u.w_u as
    the per-partition bias of the Exp eviction.  PE re-transposes ET tiles so
    DVE 3D-reduces give per-i max (q2c weight, exact) and 1/rowsum.
  - a short PE warmup burst lifts the HAM clock gate (1.2 -> 2.4 GHz) while
    the first h DMAs are in flight; per-block matmul pressure keeps it warm
    through the compute wave.
"""

import numpy as np

N_B, M_B, JX, JQ, D = 8, 1, 1024, 128, 512
P = 128
NT = JX // P    # 8 i-tiles
KC = D // P     # 4 d-chunks
TPB = 2         # tiles per block
NB = NT // TPB  # 4 blocks
IB = TPB * P    # 256 i per block

_CACHE = {}


def _build_program():
    from contextlib import ExitStack

    import concourse.bass as bass
    import concourse.tile as tile
    from concourse import bacc, mybir
    from concourse.masks import make_identity

    f32 = mybir.dt.float32
    bf16 = mybir.dt.bfloat16
    f32r = mybir.dt.float32r
    EXP = mybir.ActivationFunctionType.Exp
    IDENT_F = mybir.ActivationFunctionType.Identity
    AX = mybir.AxisListType.X
    MUL = mybir.AluOpType.mult
    ds = bass.ds

    nc = bacc.Bacc("TRN2", target_bir_lowering=False, debug=False, num_devices=8)
    h_d = nc.dram_tensor("h", [JX, D], f32, kind="ExternalInput").ap()
    u_d = nc.dram_tensor("u", [JQ, D], f32, kind="ExternalInput").ap()
    aw_d = nc.dram_tensor("alpha_w", [3 * D], f32, kind="ExternalInput").ap()
    out_d = nc.dram_tensor("out", [JX, 4 * D], f32, kind="ExternalOutput").ap()

    with tile.TileContext(nc) as tc, ExitStack() as ctx:
        consts = ctx.enter_context(tc.tile_pool(name="consts", bufs=1))
        slab = ctx.enter_context(tc.tile_pool(name="slab", bufs=3))
        # PSUM (8 banks): tp=3, s/et(+uwu,zq)=2, ua(warm+c2q+bc)=2, hap(+pt0)=1
        ps = ctx.enter_context(tc.tile_pool(name="ps", bufs=2, space="PSUM"))

        # ---- input DMAs; sync and scalar are both HWDGE issuers ----
        ident = consts.tile([P, P], f32)
        make_identity(nc, ident[:])
        w_cols = consts.tile([P, 12], f32)  # alpha_w partition-major: d = c*128+p
        nc.sync.dma_start(w_cols[:], aw_d.rearrange("(c p) -> p c", p=P))
        u_sb = consts.tile([JQ, D], f32)
        nc.sync.dma_start(u_sb[:], u_d[:])
        # h is cast to bf16 during the load (SWDGE): halves every later h op
        h_all = consts.tile([P, NT * D], bf16)  # tile t: h[t*128+p, d]
        h3 = h_all[:].rearrange("p (t d) -> p t d", t=NT)
        for q in range(4):
            nc.gpsimd.dma_start(
                h3[:, ds(2 * q, 2), :],
                h_d[ds(2 * q * P, 2 * P), :].rearrange("(t p) d -> p t d", p=P),
            )
        def o1_store(eng, q):  # h passthrough (casts bf16 back to f32)
            eng.dma_start(
                out_d[ds(2 * q * P, 2 * P), ds(0, D)].rearrange(
                    "(t p) d -> p t d", p=P
                ),
                h3[:, ds(2 * q, 2), :],
            )

        o1_store(nc.gpsimd, 0)
        o1_store(nc.gpsimd, 1)

        # ---- constants ----
        ones_row = consts.tile([1, P], f32)
        nc.vector.memset(ones_row[:], 1.0)
        ones_row16 = consts.tile([1, P], bf16)
        nc.vector.tensor_copy(ones_row16[:], ones_row[:])
        ones_col = consts.tile([P, 1], f32)
        nc.vector.memset(ones_col[:], 1.0)
        ident16 = consts.tile([P, P], bf16)
        nc.vector.tensor_copy(ident16[:], ident[:])

        # ---- u prep (high priority: the whole wave hangs off uwT) ----
        prep_hp = tc.high_priority()
        prep_hp.__enter__()
        u_r = consts.tile([JQ, D], bf16)
        nc.scalar.copy(u_r[:], u_sb[:])
        # uwT'[d, j] = u[j,d]*w_hu[d] + w_h[d]: transpose u, fold the scale
        # (w_hu) and bias (w_h) into the PSUM eviction.  The w_h term makes
        # the score matmul emit s + h.w_h directly.
        uwT = consts.tile([P, KC * JQ], bf16)
        uT_sb = consts.tile([P, KC * P], f32)
        pt0 = ps.tile([P, KC * P], f32, tag="hap", bufs=1)
        for k in range(KC):
            nc.tensor.transpose(pt0[:, ds(k * P, P)], u_sb[:, ds(k * P, P)], ident[:])
        nc.scalar.copy(uT_sb[:], pt0[:])
        for k in range(KC):
            if k % 2 == 0:
                nc.scalar.activation(
                    uwT[:, ds(k * JQ, JQ)], pt0[:, ds(k * P, P)], IDENT_F,
                    bias=w_cols[:, ds(k, 1)], scale=w_cols[:, ds(8 + k, 1)],
                )
            else:
                nc.vector.tensor_scalar(
                    uwT[:, ds(k * JQ, JQ)], pt0[:, ds(k * P, P)],
                    w_cols[:, ds(8 + k, 1)], w_cols[:, ds(k, 1)],
                    op0=MUL, op1=mybir.AluOpType.add,
                )
        # uwu[j] = sum_d u[j,d] w_u[d] via tiny fp32 matmuls on uT
        uwu = consts.tile([JQ, 1], f32)
        uwp = ps.tile([JQ, 1], f32, tag="set")
        for k in range(KC):
            nc.tensor.matmul(
                uwp[:], uT_sb[:, ds(k * P, P)], w_cols[:, ds(4 + k, 1)],
                start=(k == 0), stop=(k == KC - 1),
            )
        nc.vector.tensor_copy(uwu[:], uwp[:])
        prep_hp.__exit__(None, None, None)

        # ---- per-block streaming pipeline ----
        hT_all = consts.tile([P, KC * JX], bf16)  # chunk k: hT[k*128+p, i]
        hT3 = hT_all[:].rearrange("p (k x) -> p k x", k=KC)
        ET = consts.tile([JQ, JX], bf16)          # exp(sT + uwu + hwh)
        m_exp = consts.tile([P, NT], bf16)        # per i-tile col: max_j ET
        z_rec = consts.tile([P, NT], f32)         # per i-tile col: 1/sum_j ET
        zsum = consts.tile([P, NT], f32)
        o4_sb = consts.tile([P, NT * D], bf16)
        hap = ps.tile([1, D], f32, tag="hap", bufs=1)
        LAST = NB - 1

        for b in range(NB):
            t0 = b * TPB
            blk = ds(b * IB, IB)
            # transposes of the block's h tiles: one PSUM bank per tile
            for q in range(TPB):
                t = t0 + q
                pt = ps.tile([P, KC * P], bf16, tag="tp", bufs=3)
                for k in range(KC):
                    nc.tensor.transpose(
                        pt[:, ds(k * P, P)], h_all[:, ds(t * D + k * P, P)],
                        ident16[:]
                    )
                ev = nc.scalar.copy if q == 0 else nc.vector.tensor_copy
                ev(hT3[:, :, ds(t * P, P)], pt[:].rearrange("p (k x) -> p k x", k=KC))

            # transposed scores: sT[j, i] = sum_k uwT'_k^T @ hT_k (incl. h.w_h)
            sp = ps.tile([JQ, IB], f32, tag="set")
            for k in range(KC):
                nc.tensor.matmul(
                    sp[:], uwT[:, ds(k * JQ, JQ)],
                    hT_all[:, ds(k * JX + b * IB, IB)],
                    start=(k == 0), stop=(k == KC - 1),
                )
            # ET = exp(sT + uwu[j]) with uwu as the per-partition ACT bias
            nc.scalar.activation(ET[:, blk], sp[:], EXP, bias=uwu[:])

            # re-transpose ET tiles; batched 3D reduces -> max, 1/sum per i
            et = ps.tile([P, IB], bf16, tag="set")
            for q in range(TPB):
                t = t0 + q
                nc.tensor.transpose(et[:, ds(q * P, P)], ET[:, ds(t * P, P)],
                                    ident16[:])
            et3 = et[:].rearrange("p (q x) -> p q x", q=TPB)
            nc.vector.reduce_max(m_exp[:, ds(t0, TPB)], et3, axis=AX)
            nc.vector.reduce_sum(zsum[:, ds(t0, TPB)], et3, axis=AX)
            nc.vector.reciprocal(z_rec[:, ds(t0, TPB)], zsum[:, ds(t0, TPB)])

            # q2c accumulation (bf16; single PSUM group spanning all blocks)
            for q in range(TPB):
                t = t0 + q
                nc.tensor.matmul(
                    hap[:], m_exp[:, ds(t, 1)], h_all[:, ds(t * D, D)],
                    start=(t == 0), stop=(t == NT - 1),
                    skip_group_check=True,
                )

            if b == LAST:
                # q2c tail scalars, emitted before the last c2q so the bc
                # broadcast is ready while the slab work still streams
                tail_hp = tc.high_priority()
                tail_hp.__enter__()
                mrow = consts.tile([P, 1], f32)
                nc.vector.reduce_sum(mrow[:], m_exp[:], axis=AX)
                zqp = ps.tile([1, 1], f32, tag="set")
                nc.tensor.matmul(zqp[:], mrow[:], ones_col[:],
                                 start=True, stop=True)
                rzq = consts.tile([1, 1], f32)
                nc.vector.reciprocal(rzq[:], zqp[:])
                ha_row = consts.tile([1, D], bf16)
                nc.scalar.mul(ha_row[:], hap[:], rzq[:])
                tail_hp.__exit__(None, None, None)

            # c2q per tile: u_a = (ET_t^T @ u) * rz (scalar); o3 = u_a * h
            # (gpsimd from the slab, except the last block on DVE)
            stg = slab.tile([P, TPB * 2 * D], f32, tag="stg")
            for q in range(TPB):
                t = t0 + q
                up = ps.tile([P, D], f32, tag="ua")
                nc.tensor.matmul(up[:], ET[:, ds(t * P, P)], u_r[:],
                                 start=True, stop=True)
                o2 = stg[:, ds(q * 2 * D, D)]
                nc.scalar.mul(o2, up[:], z_rec[:, ds(t, 1)])
                if b == LAST or b == 0:
                    nc.vector.scalar_tensor_tensor(
                        stg[:, ds(q * 2 * D + D, D)], up[:], z_rec[:, ds(t, 1)],
                        h_all[:, ds(t * D, D)], op0=MUL, op1=MUL,
                    )
                else:
                    nc.gpsimd.tensor_mul(
                        stg[:, ds(q * 2 * D + D, D)], o2, h_all[:, ds(t * D, D)]
                    )
            if b == 0:
                o1_store(nc.gpsimd, 2)
                o1_store(nc.gpsimd, 3)
            nc.sync.dma_start(
                out_d[ds(b * IB, IB), ds(D, 2 * D)].rearrange(
                    "(t p) d -> p t d", p=P
                ),
                stg[:].rearrange("p (t d) -> p t d", t=TPB),
            )

        # ---- o4 = h * broadcast(h_a) ----
        bcp = ps.tile([P, D], f32, tag="ua")
        nc.tensor.matmul(bcp[:], ones_row16[:], ha_row[:], start=True, stop=True)
        bc2 = consts.tile([P, 2 * D], bf16)
        nc.scalar.copy(bc2[:, ds(0, D)], bcp[:])
        nc.vector.tensor_copy(bc2[:, ds(D, D)], bcp[:])

        for pr in range(NT // 2):
            pair = ds(2 * pr * D, 2 * D)
            nc.vector.tensor_mul(o4_sb[:, pair], h_all[:, pair], bc2[:])
            nc.gpsimd.dma_start(
                out_d[ds(2 * pr * P, 2 * P), ds(3 * D, D)].rearrange(
                    "(t p) d -> p t d", p=P
                ),
                o4_sb[:, pair].rearrange("p (t d) -> p t d", t=2),
            )

    nc.compile()
    return nc


def _get_nc():
    if "nc" not in _CACHE:
        _CACHE["nc"] = _build_program()
    return _CACHE["nc"]


def _ensure_axon_hooks_stub():
    # concourse imports antenv.axon_hooks when tracing is requested via env;
    # provide a no-op stub if the image lacks it so runs degrade gracefully.
    import sys
    import types

    try:
        import antenv.axon_hooks  # noqa: F401
    except ImportError:
        mod = types.ModuleType("antenv.axon_hooks")
        _hook = [None]
        mod.set_axon_ntff_profile_hook = lambda hook: _hook.__setitem__(0, hook)
        mod.get_axon_ntff_profile_hook = lambda: _hook[0]
        sys.modules["antenv.axon_hooks"] = mod


def kernel(h, u, alpha_w, alpha_b=None, **_unused):
    _ensure_axon_hooks_stub()
    from concourse.bass_utils import run_bass_kernel_spmd

    h = np.ascontiguousarray(np.asarray(h, dtype=np.float32)).reshape(N_B, JX, D)
    u = np.ascontiguousarray(np.asarray(u, dtype=np.float32)).reshape(N_B, JQ, D)
    alpha_w = np.ascontiguousarray(np.asarray(alpha_w, dtype=np.float32)).reshape(3 * D)

    nc = _get_nc()
    in_maps = [
        {"h": h[n], "u": u[n], "alpha_w": alpha_w} for n in range(N_B)
    ]
    res = run_bass_kernel_spmd(nc, in_maps, core_ids=list(range(N_B)))
    out = np.stack([res.results[n]["out"] for n in range(N_B)], axis=0)
    return out.reshape(N_B, M_B, JX, 4 * D)


# revision 21
# speedup vs baseline: 1.1612x; 1.1612x over previous
"""Trainium2 Bass kernel for BiDAF-style bidirectional attention.

Reference computation (per batch element n; M=1 folded away):
    s[i,j]  = h[i].w_h + u[j].w_u + (h[i]*u[j]).w_hu + b      [JX, JQ]
    a_u     = softmax_j(s);     u_a[i] = sum_j a_u[i,j] u[j]   (c2q)
    a_h     = softmax_i(max_j s);  h_a = sum_i a_h[i] h[i]     (q2c)
    out     = concat(h, u_a, h*u_a, h*h_a)                     [JX, 4D]

Sharding: data-parallel over batch N=8, one NeuronCore per batch element.
alpha_b drops out (both softmaxes are shift-invariant), accepted but unused.

Per-core I/O is 10.25 MiB (h 2 + u 0.25 + out 8), i.e. ~30 us at the 358 GB/s
per-core HBM roofline, so the schedule is built around keeping the store
stream continuous from ~4.5 us on:
  - 4 blocks of 2 i-tiles stream through scores->exp->c2q as h loads arrive
    (the c2q softmax over j is local to an i-row; only o4 = h*h_a needs the
    global max/softmax over all JX).
  - DMA count is minimized (each dma_start costs ~0.7 us of sequencer issue
    time): h as 4 x 512 KiB loads + u (sync), alpha_w broadcasts (gpsimd),
    o1 = h passthrough as 2 x 1 MiB bulk stores (gpsimd, SWDGE), per-block
    [o2|o3] slabs as 4 x 1 MiB stores and o4 as 2 x 1 MiB stores (sync).
  - f32 tiles are bitcast to f32r at matmul use sites (no cast copies).
  - scores are computed TRANSPOSED per block: sT[j,i] = sum_d uwT[d,j]hT[d,i]
    over 4 d-chunks, + h.w_h via a K=1 matmul (ones_col x hwh_row), u.w_u as
    the per-partition bias of the Exp eviction.  PE re-transposes ET tiles so
    DVE 3D-reduces give per-i max (q2c weight, exact) and 1/rowsum.
  - a short PE warmup burst lifts the HAM clock gate (1.2 -> 2.4 GHz) while
    the first h DMAs are in flight; per-block matmul pressure keeps it warm
    through the compute wave.
"""

import numpy as np

N_B, M_B, JX, JQ, D = 8, 1, 1024, 128, 512
P = 128
NT = JX // P    # 8 i-tiles
KC = D // P     # 4 d-chunks
TPB = 2         # tiles per block
NB = NT // TPB  # 4 blocks
IB = TPB * P    # 256 i per block

_CACHE = {}


def _build_program():
    from contextlib import ExitStack

    import concourse.bass as bass
    import concourse.tile as tile
    from concourse import bacc, mybir
    from concourse.masks import make_identity

    f32 = mybir.dt.float32
    bf16 = mybir.dt.bfloat16
    f32r = mybir.dt.float32r
    EXP = mybir.ActivationFunctionType.Exp
    IDENT_F = mybir.ActivationFunctionType.Identity
    AX = mybir.AxisListType.X
    MUL = mybir.AluOpType.mult
    ds = bass.ds

    nc = bacc.Bacc("TRN2", target_bir_lowering=False, debug=False, num_devices=8)
    h_d = nc.dram_tensor("h", [JX, D], f32, kind="ExternalInput").ap()
    u_d = nc.dram_tensor("u", [JQ, D], f32, kind="ExternalInput").ap()
    aw_d = nc.dram_tensor("alpha_w", [3 * D], f32, kind="ExternalInput").ap()
    out_d = nc.dram_tensor("out", [JX, 4 * D], f32, kind="ExternalOutput").ap()

    with tile.TileContext(nc) as tc, ExitStack() as ctx:
        consts = ctx.enter_context(tc.tile_pool(name="consts", bufs=1))
        slab = ctx.enter_context(tc.tile_pool(name="slab", bufs=3))
        # PSUM (8 banks): tp=3, s/et(+uwu,zq)=2, ua(warm+c2q+bc)=2, hap(+pt0)=1
        ps = ctx.enter_context(tc.tile_pool(name="ps", bufs=2, space="PSUM"))

        # ---- input DMAs; sync and scalar are both HWDGE issuers ----
        ident = consts.tile([P, P], f32)
        make_identity(nc, ident[:])
        u_sb = consts.tile([JQ, D], f32)
        nc.sync.dma_start(u_sb[:], u_d[:])
        # h is cast to bf16 during the load (SWDGE): halves every later h op
        h_all = consts.tile([P, NT * D], bf16)  # tile t: h[t*128+p, d]
        h3 = h_all[:].rearrange("p (t d) -> p t d", t=NT)
        for q in range(4):
            nc.gpsimd.dma_start(
                h3[:, ds(2 * q, 2), :],
                h_d[ds(2 * q * P, 2 * P), :].rearrange("(t p) d -> p t d", p=P),
            )
        w_cols = consts.tile([P, 12], f32)  # alpha_w partition-major: d = c*128+p
        nc.scalar.dma_start(w_cols[:], aw_d.rearrange("(c p) -> p c", p=P))

        def o1_store(eng, q):  # h passthrough (casts bf16 back to f32)
            eng.dma_start(
                out_d[ds(2 * q * P, 2 * P), ds(0, D)].rearrange(
                    "(t p) d -> p t d", p=P
                ),
                h3[:, ds(2 * q, 2), :],
            )

        o1_store(nc.gpsimd, 0)
        o1_store(nc.gpsimd, 1)

        # ---- constants ----
        ones_row = consts.tile([1, P], f32)
        nc.vector.memset(ones_row[:], 1.0)
        ones_row16 = consts.tile([1, P], bf16)
        nc.vector.tensor_copy(ones_row16[:], ones_row[:])
        ones_col = consts.tile([P, 1], f32)
        nc.vector.memset(ones_col[:], 1.0)
        ident16 = consts.tile([P, P], bf16)
        nc.vector.tensor_copy(ident16[:], ident[:])

        # ---- u prep (high priority: the whole wave hangs off uwT) ----
        prep_hp = tc.high_priority()
        prep_hp.__enter__()
        u_r = consts.tile([JQ, D], bf16)
        nc.scalar.copy(u_r[:], u_sb[:])
        # uwT'[d, j] = u[j,d]*w_hu[d] + w_h[d]: transpose u, fold the scale
        # (w_hu) and bias (w_h) into the PSUM eviction.  The w_h term makes
        # the score matmul emit s + h.w_h directly.
        uwT = consts.tile([P, KC * JQ], bf16)
        uT_sb = consts.tile([P, KC * P], f32)
        pt0 = ps.tile([P, KC * P], f32, tag="hap", bufs=1)
        for k in range(KC):
            nc.tensor.transpose(pt0[:, ds(k * P, P)], u_sb[:, ds(k * P, P)], ident[:])
        nc.scalar.copy(uT_sb[:], pt0[:])
        for k in range(KC):
            if k % 2 == 0:
                nc.scalar.activation(
                    uwT[:, ds(k * JQ, JQ)], pt0[:, ds(k * P, P)], IDENT_F,
                    bias=w_cols[:, ds(k, 1)], scale=w_cols[:, ds(8 + k, 1)],
                )
            else:
                nc.vector.tensor_scalar(
                    uwT[:, ds(k * JQ, JQ)], pt0[:, ds(k * P, P)],
                    w_cols[:, ds(8 + k, 1)], w_cols[:, ds(k, 1)],
                    op0=MUL, op1=mybir.AluOpType.add,
                )
        # uwu[j] = sum_d u[j,d] w_u[d] via tiny fp32 matmuls on uT
        uwu = consts.tile([JQ, 1], f32)
        uwp = ps.tile([JQ, 1], f32, tag="set")
        for k in range(KC):
            nc.tensor.matmul(
                uwp[:], uT_sb[:, ds(k * P, P)], w_cols[:, ds(4 + k, 1)],
                start=(k == 0), stop=(k == KC - 1),
            )
        nc.vector.tensor_copy(uwu[:], uwp[:])
        prep_hp.__exit__(None, None, None)

        # ---- per-block streaming pipeline ----
        hT_all = consts.tile([P, KC * JX], bf16)  # chunk k: hT[k*128+p, i]
        hT3 = hT_all[:].rearrange("p (k x) -> p k x", k=KC)
        ET = consts.tile([JQ, JX], bf16)          # exp(sT + uwu + hwh)
        m_exp = consts.tile([P, NT], bf16)        # per i-tile col: max_j ET
        z_rec = consts.tile([P, NT], f32)         # per i-tile col: 1/sum_j ET
        zsum = consts.tile([P, NT], f32)
        o4_sb = consts.tile([P, NT * D], f32)
        hap = ps.tile([1, D], f32, tag="hap", bufs=1)
        LAST = NB - 1

        for b in range(NB):
            t0 = b * TPB
            blk = ds(b * IB, IB)
            # transposes of the block's h tiles: one PSUM bank per tile
            for q in range(TPB):
                t = t0 + q
                pt = ps.tile([P, KC * P], bf16, tag="tp", bufs=3)
                for k in range(KC):
                    nc.tensor.transpose(
                        pt[:, ds(k * P, P)], h_all[:, ds(t * D + k * P, P)],
                        ident16[:]
                    )
                ev = nc.scalar.copy if q == 0 else nc.vector.tensor_copy
                ev(hT3[:, :, ds(t * P, P)], pt[:].rearrange("p (k x) -> p k x", k=KC))

            # transposed scores: sT[j, i] = sum_k uwT'_k^T @ hT_k (incl. h.w_h)
            sp = ps.tile([JQ, IB], f32, tag="set")
            for k in range(KC):
                nc.tensor.matmul(
                    sp[:], uwT[:, ds(k * JQ, JQ)],
                    hT_all[:, ds(k * JX + b * IB, IB)],
                    start=(k == 0), stop=(k == KC - 1),
                )
            # ET = exp(sT + uwu[j]) with uwu as the per-partition ACT bias
            nc.scalar.activation(ET[:, blk], sp[:], EXP, bias=uwu[:])

            # re-transpose ET tiles; batched 3D reduces -> max, 1/sum per i
            et = ps.tile([P, IB], bf16, tag="set")
            for q in range(TPB):
                t = t0 + q
                nc.tensor.transpose(et[:, ds(q * P, P)], ET[:, ds(t * P, P)],
                                    ident16[:])
            et3 = et[:].rearrange("p (q x) -> p q x", q=TPB)
            nc.vector.reduce_max(m_exp[:, ds(t0, TPB)], et3, axis=AX)
            nc.vector.reduce_sum(zsum[:, ds(t0, TPB)], et3, axis=AX)
            nc.vector.reciprocal(z_rec[:, ds(t0, TPB)], zsum[:, ds(t0, TPB)])

            # q2c accumulation (bf16; single PSUM group spanning all blocks)
            for q in range(TPB):
                t = t0 + q
                nc.tensor.matmul(
                    hap[:], m_exp[:, ds(t, 1)], h_all[:, ds(t * D, D)],
                    start=(t == 0), stop=(t == NT - 1),
                    skip_group_check=True,
                )

            if b == LAST:
                # q2c tail scalars, emitted before the last c2q so the bc
                # broadcast is ready while the slab work still streams
                mrow = consts.tile([P, 1], f32)
                nc.vector.reduce_sum(mrow[:], m_exp[:], axis=AX)
                zqp = ps.tile([1, 1], f32, tag="set")
                nc.tensor.matmul(zqp[:], mrow[:], ones_col[:],
                                 start=True, stop=True)
                rzq = consts.tile([1, 1], f32)
                nc.vector.reciprocal(rzq[:], zqp[:])
                ha_row = consts.tile([1, D], bf16)
                nc.scalar.mul(ha_row[:], hap[:], rzq[:])

            # c2q per tile: u_a = (ET_t^T @ u) * rz (scalar); o3 = u_a * h
            # (gpsimd from the slab, except the last block on DVE)
            stg = slab.tile([P, TPB * 2 * D], f32, tag="stg")
            for q in range(TPB):
                t = t0 + q
                up = ps.tile([P, D], f32, tag="ua")
                nc.tensor.matmul(up[:], ET[:, ds(t * P, P)], u_r[:],
                                 start=True, stop=True)
                o2 = stg[:, ds(q * 2 * D, D)]
                nc.scalar.mul(o2, up[:], z_rec[:, ds(t, 1)])
                if b == LAST or b == 0:
                    nc.vector.scalar_tensor_tensor(
                        stg[:, ds(q * 2 * D + D, D)], up[:], z_rec[:, ds(t, 1)],
                        h_all[:, ds(t * D, D)], op0=MUL, op1=MUL,
                    )
                else:
                    nc.gpsimd.tensor_mul(
                        stg[:, ds(q * 2 * D + D, D)], o2, h_all[:, ds(t * D, D)]
                    )
            if b == 0:
                o1_store(nc.gpsimd, 2)
                o1_store(nc.gpsimd, 3)
            nc.sync.dma_start(
                out_d[ds(b * IB, IB), ds(D, 2 * D)].rearrange(
                    "(t p) d -> p t d", p=P
                ),
                stg[:].rearrange("p (t d) -> p t d", t=TPB),
            )

        # ---- o4 = h * broadcast(h_a) ----
        bcp = ps.tile([P, D], f32, tag="ua")
        nc.tensor.matmul(bcp[:], ones_row16[:], ha_row[:], start=True, stop=True)
        bc2 = consts.tile([P, 2 * D], bf16)
        nc.scalar.copy(bc2[:, ds(0, D)], bcp[:])
        nc.vector.tensor_copy(bc2[:, ds(D, D)], bcp[:])

        for pr in range(NT // 2):
            pair = ds(2 * pr * D, 2 * D)
            nc.vector.tensor_mul(o4_sb[:, pair], h_all[:, pair], bc2[:])
            nc.sync.dma_start(
                out_d[ds(2 * pr * P, 2 * P), ds(3 * D, D)].rearrange(
                    "(t p) d -> p t d", p=P
                ),
                o4_sb[:, pair].rearrange("p (t d) -> p t d", t=2),
            )

    nc.compile()
    return nc


def _get_nc():
    if "nc" not in _CACHE:
        _CACHE["nc"] = _build_program()
    return _CACHE["nc"]


def _ensure_axon_hooks_stub():
    # concourse imports antenv.axon_hooks when tracing is requested via env;
    # provide a no-op stub if the image lacks it so runs degrade gracefully.
    import sys
    import types

    try:
        import antenv.axon_hooks  # noqa: F401
    except ImportError:
        mod = types.ModuleType("antenv.axon_hooks")
        _hook = [None]
        mod.set_axon_ntff_profile_hook = lambda hook: _hook.__setitem__(0, hook)
        mod.get_axon_ntff_profile_hook = lambda: _hook[0]
        sys.modules["antenv.axon_hooks"] = mod


def kernel(h, u, alpha_w, alpha_b=None, **_unused):
    _ensure_axon_hooks_stub()
    from concourse.bass_utils import run_bass_kernel_spmd

    h = np.ascontiguousarray(np.asarray(h, dtype=np.float32)).reshape(N_B, JX, D)
    u = np.ascontiguousarray(np.asarray(u, dtype=np.float32)).reshape(N_B, JQ, D)
    alpha_w = np.ascontiguousarray(np.asarray(alpha_w, dtype=np.float32)).reshape(3 * D)

    nc = _get_nc()
    in_maps = [
        {"h": h[n], "u": u[n], "alpha_w": alpha_w} for n in range(N_B)
    ]
    res = run_bass_kernel_spmd(nc, in_maps, core_ids=list(range(N_B)))
    out = np.stack([res.results[n]["out"] for n in range(N_B)], axis=0)
    return out.reshape(N_B, M_B, JX, 4 * D)
